# revision 1
# baseline (speedup 1.0000x reference)
"""Llama4 MoE (T=1024, H=1024, I=2048, SI=4096, E=8, K=1) on 8 trn2 NeuronCores.

Sharding (expert-parallel + shared-TP, host-side combine):
  - core c gets expert c's gate/up/down weights (full), a 512-wide slice of the
    shared expert (columns of shared_gate/up, rows of shared_down), the full
    hidden_states and router weight.
  - Each core computes router logits + top-1 combine weights for ALL tokens,
    compacts its expert's tokens into C=256 capacity slots with a
    permutation-matrix matmul on the tensor engine (gather fused with the
    router-weight scaling), runs the expert MLP at N=256, scatters the result
    back to token positions with the transposed permutation, adds its shared
    shard, and writes a partial output outT[H, T] (transposed layout).
  - Host: out = (sum_c outT_c).T    (sum over cores = expert sum + shared TP
    all-reduce; transpose restores [T, H]).

Everything works in transposed layout (features on partitions) so all weight
matrices stream from HBM in natural row-major layout. Big matmuls run in
float32r (single-pass fp32, 4x faster than double-pumped fp32, ~1e-4 rel
error); the router logits stay exact fp32 so argmax matches the fp32
reference bit-for-bit.
"""

import functools
import numpy as np

T, H, I, SI, E = 1024, 1024, 2048, 4096, 8
NCORES = 8
SIS = SI // NCORES  # 512: shared intermediate shard per core
P = 128
C = 256        # expert token capacity (mean load 128, sigma ~10.6)
HO = H // P    # 8  k-subtiles over hidden
TT = T // P    # 8  token tiles
IT = I // P    # 16 routed-intermediate tiles
ST = SIS // P  # 4  shared-shard tiles
NH = 2         # token halves (moving-operand free dim 512)
NF = T // NH   # 512
BIG = 20000.0  # out-of-range slot for unselected tokens


def _build_nc():
    import concourse.mybir as mybir
    import concourse.tile as tile
    from concourse import bacc
    from concourse.masks import make_identity

    F32 = mybir.dt.float32
    F32R = mybir.dt.float32r
    AF = mybir.ActivationFunctionType
    ALU = mybir.AluOpType
    AX = mybir.AxisListType
    R = lambda ap: ap.bitcast(F32R)

    nc = bacc.Bacc(trn_type="TRN2")

    x_d = nc.dram_tensor("x", [T, H], F32, kind="ExternalInput")
    rwt_d = nc.dram_tensor("rwt", [H, E], F32, kind="ExternalInput")
    esel_d = nc.dram_tensor("esel", [P, E], F32, kind="ExternalInput")
    iotac_d = nc.dram_tensor("iotac", [P, C], F32, kind="ExternalInput")
    iotaj_d = nc.dram_tensor("iotaj", [P, C // P], F32, kind="ExternalInput")
    ltri_d = nc.dram_tensor("ltri", [P, P], F32, kind="ExternalInput")
    sg_d = nc.dram_tensor("sgate", [H, SIS], F32, kind="ExternalInput")
    su_d = nc.dram_tensor("sup", [H, SIS], F32, kind="ExternalInput")
    sd_d = nc.dram_tensor("sdown", [SIS, H], F32, kind="ExternalInput")
    eg_d = nc.dram_tensor("egate", [H, I], F32, kind="ExternalInput")
    eu_d = nc.dram_tensor("eup", [H, I], F32, kind="ExternalInput")
    ed_d = nc.dram_tensor("edown", [I, H], F32, kind="ExternalInput")
    out_d = nc.dram_tensor("outT", [H, T], F32, kind="ExternalOutput")

    with tile.TileContext(nc) as tc:
        with (
            tc.tile_pool(name="persist", bufs=1) as pp,
            tc.tile_pool(name="xin", bufs=3) as xp,
            tc.tile_pool(name="wstream", bufs=5) as wp,
            tc.tile_pool(name="outst", bufs=3) as op,
            tc.tile_pool(name="small", bufs=2) as sp,
            tc.tile_pool(name="ps_small", bufs=2, space="PSUM") as ps_s,
            tc.tile_pool(name="ps_mm", bufs=5, space="PSUM") as ps_mm,
        ):
            # ---- constants ----
            ident = pp.tile([P, P], F32, tag="ident", name="ident")
            make_identity(nc, ident)
            # fp32r-typed identity for transposes of fp32r data (the
            # verifier requires fp32r consumers to have fp32r producers)
            identr = pp.tile([P, P], F32R, tag="identr", name="identr")
            nc.vector.tensor_copy(identr, ident)
            # sel[:, tt*P:(tt+1)*P] has row tt = 1.0: lhsT that broadcasts
            # row tt of an [TT, P] rhs across all 128 output partitions.
            sel_sb = pp.tile([TT, TT * P], F32, tag="sel", name="sel_sb")
            for tt in range(TT):
                nc.vector.tensor_copy(
                    sel_sb[:, tt * P:(tt + 1) * P],
                    ident[:TT, tt:tt + 1].to_broadcast([TT, P]))
            allones8 = pp.tile([TT, P], F32, tag="allones8", name="allones8")
            nc.vector.memset(allones8, 1.0)
            onescol = pp.tile([P, 1], F32, tag="onescol", name="onescol")
            nc.vector.memset(onescol, 1.0)
            rwT = pp.tile([P, HO, E], F32, tag="rwT", name="rwT")

            # ---- x load + transpose + router logits ----
            xT = pp.tile([P, HO, T], F32R, tag="xT", name="xT")
            L_sb = pp.tile([P, TT, E], F32, tag="L", name="L_sb")
            xr_tiles = []
            xt_tiles = []
            for tt in range(TT):
                x_t = xp.tile([P, H], F32, tag="x_t", name="x_t")
                nc.sync.dma_start(x_t, x_d[tt * P:(tt + 1) * P, :])
                if tt == 0:
                    nc.sync.dma_start(
                        rwT, rwt_d[:].rearrange("(ko p) e -> p ko e", p=P))
                psL = ps_s.tile([P, E], F32, tag="psL", name="psL", bufs=1)
                for kg in range(2):  # 4 transposes batched per psum bank
                    pst = ps_s.tile([P, 4, P], F32, tag="ps_sm", name="pst_x")
                    for kj in range(4):
                        ko = kg * 4 + kj
                        nc.tensor.transpose(pst[:, kj, :],
                                            x_t[:, ko * P:(ko + 1) * P], ident)
                    # rounded copy feeds the big fp32r matmuls
                    nc.vector.tensor_copy(
                        xT[:, kg * 4:(kg + 1) * 4, tt * P:(tt + 1) * P], pst)
                    # exact fp32 staging feeds the router so argmax matches
                    # the fp32 reference bit-for-bit
                    xst = xp.tile([P, 4, P], F32, tag="xst", name="xst", bufs=2)
                    nc.vector.tensor_copy(xst, pst)
                    for kj in range(4):
                        ko = kg * 4 + kj
                        nc.tensor.matmul(psL, xst[:, kj, :], rwT[:, ko, :],
                                         start=(ko == 0), stop=(ko == HO - 1))
                nc.vector.tensor_copy(L_sb[:, tt, :], psL)
                xt_tiles.append(x_t)

            esel_sb = pp.tile([P, E], F32, tag="esel", name="esel_sb")
            nc.sync.dma_start(esel_sb, esel_d[:, :])
            iotac = pp.tile([P, C], F32, tag="iotac", name="iotac")
            nc.sync.dma_start(iotac, iotac_d[:, :])
            iotaj = pp.tile([P, C // P], F32, tag="iotaj", name="iotaj")
            nc.sync.dma_start(iotaj, iotaj_d[:, :])
            ltri = pp.tile([P, P], F32, tag="ltri", name="ltri")
            nc.sync.dma_start(ltri, ltri_d[:, :])

            # ---- top-1 combine: mask m and weight combw, both [t_p, tt] ----
            maxc = sp.tile([P, TT], F32, tag="maxc", name="maxc")
            nc.vector.reduce_max(maxc, L_sb, axis=AX.X)
            w_sb = sp.tile([P, TT], F32, tag="wsb", name="w_sb")
            nc.scalar.activation(w_sb, maxc, AF.Sigmoid)
            eq = sp.tile([P, TT, E], F32, tag="eq", name="eq")
            nc.vector.tensor_tensor(eq, L_sb,
                                    maxc[:, :, None].to_broadcast([P, TT, E]),
                                    ALU.is_equal)
            nc.vector.tensor_tensor(eq, eq,
                                    esel_sb[:, None, :].to_broadcast([P, TT, E]),
                                    ALU.mult)
            m_sb = sp.tile([P, TT], F32, tag="m", name="m_sb")
            nc.vector.reduce_sum(m_sb, eq, axis=AX.X)
            combw = sp.tile([P, TT], F32, tag="combw", name="combw")
            nc.vector.tensor_tensor(combw, m_sb, w_sb, ALU.mult)

            # ---- shared expert gate/up on xT -> gsT[si_p, st, t] ----
            gsT = pp.tile([P, ST, T], F32R, tag="gsT", name="gsT")
            for sb in range(1):  # first shared slab pair
                sg_sl = wp.tile([P, HO, 256], F32R, tag="w8", name="sg_sl")
                nc.sync.dma_start(
                    sg_sl, R(sg_d[:]).rearrange("(ko p) i -> p ko i", p=P)
                    [:, :, sb * 256:(sb + 1) * 256])
                su_sl = wp.tile([P, HO, 256], F32R, tag="w8", name="su_sl")
                nc.sync.dma_start(
                    su_sl, R(su_d[:]).rearrange("(ko p) i -> p ko i", p=P)
                    [:, :, sb * 256:(sb + 1) * 256])
                for a in range(2):
                    si = sb * 2 + a
                    for nh in range(NH):
                        nsl = slice(nh * NF, (nh + 1) * NF)
                        psg = ps_mm.tile([P, NF], F32, tag="ps_mm", name="psg")
                        for ko in range(HO):
                            nc.tensor.matmul(psg,
                                             sg_sl[:, ko, a * P:(a + 1) * P],
                                             xT[:, ko, nsl],
                                             start=(ko == 0),
                                             stop=(ko == HO - 1))
                        psu = ps_mm.tile([P, NF], F32, tag="ps_mm", name="psu")
                        for ko in range(HO):
                            nc.tensor.matmul(psu,
                                             su_sl[:, ko, a * P:(a + 1) * P],
                                             xT[:, ko, nsl],
                                             start=(ko == 0),
                                             stop=(ko == HO - 1))
                        # silu(g) * u == sigmoid(g) * g * u
                        nc.scalar.activation(gsT[:, si, nsl], psg, AF.Sigmoid)
                        nc.vector.tensor_tensor(gsT[:, si, nsl],
                                                gsT[:, si, nsl], psg, ALU.mult)
                        nc.vector.tensor_tensor(gsT[:, si, nsl],
                                                gsT[:, si, nsl], psu, ALU.mult)

            # ---- shared expert gate/up, second half ----
            for sb in range(1, 2):  # second shared slab pair
                sg_sl = wp.tile([P, HO, 256], F32R, tag="w8", name="sg_sl")
                nc.sync.dma_start(
                    sg_sl, R(sg_d[:]).rearrange("(ko p) i -> p ko i", p=P)
                    [:, :, sb * 256:(sb + 1) * 256])
                su_sl = wp.tile([P, HO, 256], F32R, tag="w8", name="su_sl")
                nc.sync.dma_start(
                    su_sl, R(su_d[:]).rearrange("(ko p) i -> p ko i", p=P)
                    [:, :, sb * 256:(sb + 1) * 256])
                for a in range(2):
                    si = sb * 2 + a
                    for nh in range(NH):
                        nsl = slice(nh * NF, (nh + 1) * NF)
                        psg = ps_mm.tile([P, NF], F32, tag="ps_mm", name="psg")
                        for ko in range(HO):
                            nc.tensor.matmul(psg,
                                             sg_sl[:, ko, a * P:(a + 1) * P],
                                             xT[:, ko, nsl],
                                             start=(ko == 0),
                                             stop=(ko == HO - 1))
                        psu = ps_mm.tile([P, NF], F32, tag="ps_mm", name="psu")
                        for ko in range(HO):
                            nc.tensor.matmul(psu,
                                             su_sl[:, ko, a * P:(a + 1) * P],
                                             xT[:, ko, nsl],
                                             start=(ko == 0),
                                             stop=(ko == HO - 1))
                        # silu(g) * u == sigmoid(g) * g * u
                        nc.scalar.activation(gsT[:, si, nsl], psg, AF.Sigmoid)
                        nc.vector.tensor_tensor(gsT[:, si, nsl],
                                                gsT[:, si, nsl], psg, ALU.mult)
                        nc.vector.tensor_tensor(gsT[:, si, nsl],
                                                gsT[:, si, nsl], psu, ALU.mult)

            # ---- capacity slots: slot[t] = #selected tokens before t ----
            # within-tile exclusive cumsum over the partition (token) axis
            ps_cs = ps_s.tile([P, TT], F32, tag="psL", name="ps_cs", bufs=1)
            nc.tensor.matmul(ps_cs, ltri, m_sb, start=True, stop=True)
            # per-tile totals, tt on partitions: sumsT[tt, 0]
            ps_sm2 = ps_s.tile([TT, 1], F32, tag="ps_sm", name="ps_sm2")
            nc.tensor.matmul(ps_sm2, m_sb, onescol, start=True, stop=True)
            sumsT = sp.tile([TT, 1], F32, tag="sumsT", name="sumsT")
            nc.vector.tensor_copy(sumsT, ps_sm2)
            # LS[k, tt] = sums[k] * (k < tt)   (strict lower 8x8 from ltri)
            LS = sp.tile([TT, TT], F32, tag="LS", name="LS")
            nc.vector.tensor_tensor(LS, ltri[:TT, :TT],
                                    sumsT.to_broadcast([TT, TT]), ALU.mult)
            # offB[p, tt] = sum_k LS[k, tt]  (same value on all partitions)
            ps_off = ps_s.tile([P, TT], F32, tag="ps_sm", name="ps_off")
            nc.tensor.matmul(ps_off, allones8, LS, start=True, stop=True)
            slot = sp.tile([P, TT], F32, tag="slot", name="slot")
            nc.vector.tensor_copy(slot, ps_cs)
            nc.vector.tensor_tensor(slot, slot, ps_off, ALU.add)
            # unselected tokens get an out-of-range slot
            slotm = sp.tile([P, TT], F32, tag="slotm", name="slotm")
            nc.vector.tensor_tensor(slotm, slot, m_sb, ALU.mult)
            inv = sp.tile([P, TT], F32, tag="inv", name="inv")
            nc.vector.tensor_scalar(inv, m_sb, -BIG, BIG, ALU.mult, ALU.add)
            nc.vector.tensor_tensor(slotm, slotm, inv, ALU.add)

            # ---- gather permutation Perm[t_p, tt, j] = combw * (slot==j) ----
            perm = pp.tile([P, TT, C], F32R, tag="perm", name="perm")
            for tt in range(TT):
                nc.vector.tensor_tensor(
                    perm[:, tt, :],
                    slotm[:, tt:tt + 1].to_broadcast([P, C]),
                    iotac, ALU.is_equal)
                nc.vector.tensor_tensor(
                    perm[:, tt, :], perm[:, tt, :],
                    combw[:, tt:tt + 1].to_broadcast([P, C]), ALU.mult)

            # fp32r copies of the raw x rows for the gather matmuls (on the
            # idle scalar engine so the DVE comb/perm chain isn't delayed)
            for tt in range(TT):
                x_r = pp.tile([P, H], F32R, tag=f"x_r{tt}", name="x_r")
                nc.scalar.activation(x_r, xt_tiles[tt], AF.Copy)
                xr_tiles.append(x_r)

            # ---- scatter permutation PermT[j_p, jo, t] = (slot[t]==j) ----
            # slot row vector: transpose slotm then broadcast via sel matmul
            ps_st = ps_s.tile([TT, P], F32, tag="ps_sm", name="ps_st")
            nc.tensor.transpose(ps_st, slotm, ident)
            st_sb = sp.tile([TT, P], F32, tag="st", name="st_sb")
            nc.vector.tensor_copy(st_sb, ps_st)
            slotB = pp.tile([P, T], F32, tag="slotB", name="slotB")
            for nh in range(NH):
                psb = ps_mm.tile([P, NF], F32, tag="ps_mm", name="psb")
                for tj in range(TT // NH):
                    tt = nh * (TT // NH) + tj
                    nc.tensor.matmul(psb[:, tj * P:(tj + 1) * P],
                                     sel_sb[:, tt * P:(tt + 1) * P], st_sb,
                                     start=True, stop=True)
                nc.vector.tensor_copy(slotB[:, nh * NF:(nh + 1) * NF], psb)
            permT = pp.tile([P, C // P, T], F32R, tag="permT", name="permT")
            for jo in range(C // P):
                nc.vector.tensor_tensor(
                    permT[:, jo, :], slotB,
                    iotaj[:, jo:jo + 1].to_broadcast([P, T]), ALU.is_equal)

            # ---- gather: xeT[h_p, ho, j] = sum_t x[t, h]*Perm[t, j] ----
            xeT = pp.tile([P, HO, C], F32R, tag="xeT", name="xeT")
            for ho in range(HO):
                psx = ps_mm.tile([P, C], F32, tag="ps_mm", name="psx")
                for tt in range(TT):
                    nc.tensor.matmul(psx,
                                     xr_tiles[tt][:, ho * P:(ho + 1) * P],
                                     perm[:, tt, :],
                                     start=(tt == 0), stop=(tt == TT - 1))
                nc.vector.tensor_copy(xeT[:, ho, :], psx)

            # ---- routed expert gate/up at capacity C -> gTe[i_p, it, j] ----
            gTe = pp.tile([P, IT, C], F32R, tag="gTe", name="gTe")
            for ib in range(I // 256):  # 8 slabs of 256 intermediate cols
                eg_sl = wp.tile([P, HO, 256], F32R, tag="w8", name="eg_sl")
                nc.sync.dma_start(
                    eg_sl, R(eg_d[:]).rearrange("(ko p) i -> p ko i", p=P)
                    [:, :, ib * 256:(ib + 1) * 256])
                eu_sl = wp.tile([P, HO, 256], F32R, tag="w8", name="eu_sl")
                nc.sync.dma_start(
                    eu_sl, R(eu_d[:]).rearrange("(ko p) i -> p ko i", p=P)
                    [:, :, ib * 256:(ib + 1) * 256])
                for a in range(2):
                    it = ib * 2 + a
                    psg = ps_mm.tile([P, C], F32, tag="ps_mm", name="psg2")
                    for ko in range(HO):
                        nc.tensor.matmul(psg,
                                         eg_sl[:, ko, a * P:(a + 1) * P],
                                         xeT[:, ko, :],
                                         start=(ko == 0), stop=(ko == HO - 1))
                    psu = ps_mm.tile([P, C], F32, tag="ps_mm", name="psu2")
                    for ko in range(HO):
                        nc.tensor.matmul(psu,
                                         eu_sl[:, ko, a * P:(a + 1) * P],
                                         xeT[:, ko, :],
                                         start=(ko == 0), stop=(ko == HO - 1))
                    nc.scalar.activation(gTe[:, it, :], psg, AF.Sigmoid)
                    nc.vector.tensor_tensor(gTe[:, it, :], gTe[:, it, :],
                                            psg, ALU.mult)
                    nc.vector.tensor_tensor(gTe[:, it, :], gTe[:, it, :],
                                            psu, ALU.mult)

            # ---- routed down at capacity C, then transpose to reJ[j_p, h] ----
            reJ = pp.tile([P, C // P, H], F32R, tag="reJ", name="reJ")
            for ho in range(HO):
                ed_sl = wp.tile([P, IT, P], F32R, tag="w8", name="ed_sl")
                nc.sync.dma_start(
                    ed_sl, R(ed_d[:]).rearrange("(ko p) h -> p ko h", p=P)
                    [:, :, ho * P:(ho + 1) * P])
                psd = ps_mm.tile([P, C], F32, tag="ps_mm", name="psd")
                for ik in range(IT):
                    nc.tensor.matmul(psd, ed_sl[:, ik, :], gTe[:, ik, :],
                                     start=(ik == 0), stop=(ik == IT - 1))
                re_sb = op.tile([P, C], F32R, tag="re", name="re_sb")
                nc.vector.tensor_copy(re_sb, psd)
                for jo in range(C // P):
                    ps_tr = ps_s.tile([P, P], F32R, tag="ps_sm", name="ps_tr")
                    nc.tensor.transpose(ps_tr, re_sb[:, jo * P:(jo + 1) * P],
                                        identr)
                    nc.vector.tensor_copy(reJ[:, jo, ho * P:(ho + 1) * P],
                                          ps_tr)

            # ---- scatter + shared down -> outT[h_p, t] ----
            for hb in range(2):  # sdown slabs over 512 output cols
                sd_sl = wp.tile([P, ST, 512], F32R, tag="w8", name="sd_sl")
                nc.sync.dma_start(
                    sd_sl, R(sd_d[:]).rearrange("(ko p) h -> p ko h", p=P)
                    [:, :, hb * 512:(hb + 1) * 512])
                for hj in range(4):
                    ho = hb * 4 + hj
                    for nh in range(NH):
                        nsl = slice(nh * NF, (nh + 1) * NF)
                        psd2 = ps_mm.tile([P, NF], F32, tag="ps_mm",
                                          name="psd2")
                        for jo in range(C // P):
                            nc.tensor.matmul(psd2,
                                             reJ[:, jo, ho * P:(ho + 1) * P],
                                             permT[:, jo, nsl],
                                             start=(jo == 0), stop=False)
                        for sk in range(ST):
                            nc.tensor.matmul(psd2,
                                             sd_sl[:, sk, hj * P:(hj + 1) * P],
                                             gsT[:, sk, nsl],
                                             start=False, stop=(sk == ST - 1))
                        o_t = op.tile([P, NF], F32, tag="ot", name="o_t")
                        nc.vector.tensor_copy(o_t, psd2)
                        nc.sync.dma_start(out_d[ho * P:(ho + 1) * P, nsl], o_t)

    nc.compile()
    return nc


@functools.lru_cache(maxsize=1)
def _get_nc():
    return _build_nc()


def _make_in_maps(inputs):
    f = lambda v: np.ascontiguousarray(np.asarray(v), dtype=np.float32)
    x = f(inputs["hidden_states"])
    rw = f(inputs["router_weight"])
    sg = f(inputs["shared_gate"])
    su = f(inputs["shared_up"])
    sd = f(inputs["shared_down"])
    eg = f(inputs["expert_gate"])
    eu = f(inputs["expert_up"])
    ed = f(inputs["expert_down"])
    iotac = np.tile(np.arange(C, dtype=np.float32), (P, 1))
    iotaj = (np.arange(P, dtype=np.float32)[:, None]
             + P * np.arange(C // P, dtype=np.float32)[None, :])
    # ltri[t', t] = 1 iff t' < t  (strict upper in row-major = lhsT layout)
    ltri = np.triu(np.ones((P, P), dtype=np.float32), 1)
    in_maps = []
    for c in range(NCORES):
        esel = np.zeros((P, E), dtype=np.float32)
        esel[:, c] = 1.0
        in_maps.append({
            "x": x,
            "rwt": np.ascontiguousarray(rw.T),
            "esel": esel,
            "iotac": iotac,
            "iotaj": np.ascontiguousarray(iotaj),
            "ltri": ltri,
            "sgate": np.ascontiguousarray(sg[:, c * SIS:(c + 1) * SIS]),
            "sup": np.ascontiguousarray(su[:, c * SIS:(c + 1) * SIS]),
            "sdown": np.ascontiguousarray(sd[c * SIS:(c + 1) * SIS, :]),
            "egate": np.ascontiguousarray(eg[c]),
            "eup": np.ascontiguousarray(eu[c]),
            "edown": np.ascontiguousarray(ed[c]),
        })
    return in_maps


def _run(inputs, trace=False):
    from concourse.bass_utils import run_bass_kernel_spmd
    nc = _get_nc()
    in_maps = _make_in_maps(inputs)
    res = run_bass_kernel_spmd(nc, in_maps, core_ids=list(range(NCORES)),
                               trace=trace)
    acc = np.zeros((H, T), dtype=np.float64)
    for r in res.results:
        acc += r["outT"].astype(np.float64)
    out = np.ascontiguousarray(acc.T).astype(np.float32)
    return out, res


def kernel(**inputs) -> np.ndarray:
    out, _ = _run(inputs, trace=False)
    return out



# revision 34
# speedup vs baseline: 1.6924x; 1.6924x over previous
"""Llama4 MoE (T=1024, H=1024, I=2048, SI=4096, E=8, K=1) on 8 trn2 NeuronCores.

Sharding (expert-parallel + shared-TP, host-side combine):
  - core c owns expert c (full gate/up/down) plus a 512-wide slice of the
    shared expert. Every core routes all tokens (cheap), compacts its
    expert's tokens into C capacity slots, runs the expert MLP at width C,
    and scatter-adds the result back to token rows.
  - Host: out = sum_c (shared_partial_c + routed_c).

Everything runs in bf16 on the PE (1 cycle/row at any free size; halves HBM
traffic vs fp32). The router is computed as xb@wb + xb@wr + xr@wb where
xb/wb are bf16 roundings and xr/wr bf16 residuals: max logit error ~2e-5
vs fp32, far below the minimum top-2 logit gap (~3e-4), so the argmax
matches the fp32 reference exactly.

Token dispatch uses the SWDGE DMA-gather (transpose mode): it gathers the
selected token rows from DRAM and writes them already transposed as
[h_part, ho, slot] -- zero tensor-engine cost. The return scatter uses the
SWDGE DMA scatter-add into a zero-initialized DRAM output.

Capacity C is chosen at runtime from the actual expert loads (host numpy
router), rounded up; the compiled program is cached per C.
"""

import functools
import numpy as np
import ml_dtypes

BF = ml_dtypes.bfloat16

T, H, I, SI, E = 1024, 1024, 2048, 4096, 8
NCORES = 8
SIS = SI // NCORES  # 512
P = 128
HO = H // P         # 8
TT = T // P         # 8
IT = I // P         # 16
ST = SIS // P       # 4
NQ = 4              # token quarters for shared gate/up
QF = T // NQ        # 256
NH = 2              # token halves for shared down
NF = T // NH        # 512
CPAD = 256          # gather width (must be a multiple of 128)
BIG = 20000.0       # out-of-range slot for unselected tokens


def _build_nc(C):
    """C: expert token capacity (multiple of 16, <= CPAD)."""
    import concourse.mybir as mybir
    import concourse.tile as tile
    from concourse import bacc
    from concourse.masks import make_identity

    F32 = mybir.dt.float32
    BF16 = mybir.dt.bfloat16
    I16 = mybir.dt.int16
    AF = mybir.ActivationFunctionType
    ALU = mybir.AluOpType
    AX = mybir.AxisListType

    CE = C + 16   # scatter entries: first 16 sacrificial (entry-0 drop)
    CB = (CE + P - 1) // P       # entry partition-blocks for scatter source
    CI = CE // 16                # index columns used by the scatter

    nc = bacc.Bacc(trn_type="TRN2")

    xtb_d = nc.dram_tensor("xtb", [P, HO, T], BF16, kind="ExternalInput")
    xtr_d = nc.dram_tensor("xtr", [P, HO, T], BF16, kind="ExternalInput")
    xnat_d = nc.dram_tensor("xnat", [T, H], BF16, kind="ExternalInput")
    rwb_d = nc.dram_tensor("rwb", [P, HO, E], BF16, kind="ExternalInput")
    rwr_d = nc.dram_tensor("rwr", [P, HO, E], BF16, kind="ExternalInput")
    sgb_d = nc.dram_tensor("sgb", [P, HO, SIS], BF16, kind="ExternalInput")
    sub_d = nc.dram_tensor("sub", [P, HO, SIS], BF16, kind="ExternalInput")
    sdb_d = nc.dram_tensor("sdb", [P, ST, H], BF16, kind="ExternalInput")
    egb_d = nc.dram_tensor("egb", [P, HO, I], BF16, kind="ExternalInput")
    eub_d = nc.dram_tensor("eub", [P, HO, I], BF16, kind="ExternalInput")
    edb_d = nc.dram_tensor("edb", [P, IT, H], BF16, kind="ExternalInput")
    iotac_d = nc.dram_tensor("iotac", [P, CPAD], F32, kind="ExternalInput")
    iotag_d = nc.dram_tensor("iotag", [P, 16], F32, kind="ExternalInput")
    iotam_d = nc.dram_tensor("iotam", [P, P], F32, kind="ExternalInput")
    iotasm1_d = nc.dram_tensor("iotasm1", [P, 16], F32, kind="ExternalInput")
    iotat1_d = nc.dram_tensor("iotat1", [P, TT], F32, kind="ExternalInput")
    iotat_d = nc.dram_tensor("iotat", [P, TT], F32, kind="ExternalInput")
    esel_d = nc.dram_tensor("esel", [P, E], F32, kind="ExternalInput")
    ltri_d = nc.dram_tensor("ltri", [P, P], F32, kind="ExternalInput")
    outsh_d = nc.dram_tensor("outsh", [T, H], BF16, kind="ExternalOutput")
    # scatter-add target, +1 row shifted: the SWDGE scatter drops index 0,
    # so rows land at token+1 and the host slices [1:]
    routed_d = nc.dram_tensor("routed", [T + 1, H], BF16,
                              kind="ExternalOutput")
    import os
    DBG = os.environ.get("KDBG") == "1"
    if DBG:
        dbg_xe_d = nc.dram_tensor("dbg_xe", [P, HO, CPAD], BF16,
                                  kind="ExternalOutput")
        dbg_cw_d = nc.dram_tensor("dbg_cw", [P, CPAD], BF16,
                                  kind="ExternalOutput")
        dbg_idx_d = nc.dram_tensor("dbg_idx", [P, CPAD // 16], I16,
                                   kind="ExternalOutput")
        dbg_sc_d = nc.dram_tensor("dbg_sc", [P, (CPAD + P - 1) // P, H],
                                  BF16, kind="ExternalOutput")

    with tile.TileContext(nc) as tc:
        with (
            tc.tile_pool(name="persist", bufs=1) as pp,
            tc.tile_pool(name="wstream", bufs=2) as wp,
            tc.tile_pool(name="outst", bufs=3) as op,
            tc.tile_pool(name="ps_big", bufs=2, space="PSUM") as ps_b,
            tc.tile_pool(name="ps_sm", bufs=2, space="PSUM") as ps_s,
        ):
            # ---- constants ----
            ident = pp.tile([P, P], F32, tag="ident", name="ident")
            make_identity(nc, ident)
            identb = pp.tile([P, P], BF16, tag="identb", name="identb")
            nc.vector.tensor_copy(identb, ident)
            onesb = pp.tile([P, P], BF16, tag="onesb", name="onesb")
            nc.vector.memset(onesb, 1.0)
            onescol = pp.tile([P, 1], F32, tag="onescol", name="onescol")
            nc.vector.memset(onescol, 1.0)
            allones8 = pp.tile([TT, P], F32, tag="allones8", name="allones8")
            nc.vector.memset(allones8, 1.0)

            # ---- early DMAs: x quarters + shared weight slabs ----
            xtb = pp.tile([P, HO, T], BF16, tag="xtb", name="xtb")
            xtr = pp.tile([P, HO, T], BF16, tag="xtr", name="xtr")
            sg_sb = pp.tile([P, HO, SIS], BF16, tag="sg", name="sg_sb")
            su_sb = pp.tile([P, HO, SIS], BF16, tag="su", name="su_sb")
            rwb = pp.tile([P, HO, E], BF16, tag="rwb", name="rwb")
            rwr = pp.tile([P, HO, E], BF16, tag="rwr", name="rwr")

            nc.sync.dma_start(xtb[:, :, 0 * QF:1 * QF], xtb_d[:, :, 0 * QF:1 * QF])
            nc.sync.dma_start(sg_sb[:, :, 0:256], sgb_d[:, :, 0:256])
            nc.sync.dma_start(su_sb[:, :, 0:256], sub_d[:, :, 0:256])
            nc.sync.dma_start(xtb[:, :, 1 * QF:2 * QF], xtb_d[:, :, 1 * QF:2 * QF])
            nc.sync.dma_start(rwb, rwb_d[:])
            nc.sync.dma_start(rwr, rwr_d[:])
            nc.sync.dma_start(xtr[:, :, 0:NF], xtr_d[:, :, 0:NF])
            nc.sync.dma_start(sg_sb[:, :, 256:512], sgb_d[:, :, 256:512])
            nc.sync.dma_start(su_sb[:, :, 256:512], sub_d[:, :, 256:512])
            nc.sync.dma_start(xtb[:, :, 2 * QF:3 * QF], xtb_d[:, :, 2 * QF:3 * QF])
            nc.sync.dma_start(xtb[:, :, 3 * QF:4 * QF], xtb_d[:, :, 3 * QF:4 * QF])
            nc.sync.dma_start(xtr[:, :, NF:T], xtr_d[:, :, NF:T])

            # small constants
            iotac = pp.tile([P, CPAD], F32, tag="iotac", name="iotac")
            nc.sync.dma_start(iotac, iotac_d[:])
            iotag = pp.tile([P, 16], F32, tag="iotag", name="iotag")
            nc.sync.dma_start(iotag, iotag_d[:])
            iotam = pp.tile([P, P], F32, tag="iotam", name="iotam")
            nc.sync.dma_start(iotam, iotam_d[:])
            iotasm1 = pp.tile([P, 16], F32, tag="iotasm1", name="iotasm1")
            nc.sync.dma_start(iotasm1, iotasm1_d[:])
            iotat1 = pp.tile([P, TT], F32, tag="iotat1", name="iotat1")
            nc.sync.dma_start(iotat1, iotat1_d[:])
            iotat = pp.tile([P, TT], F32, tag="iotat", name="iotat")
            nc.sync.dma_start(iotat, iotat_d[:])
            esel_sb = pp.tile([P, E], F32, tag="esel", name="esel_sb")
            nc.sync.dma_start(esel_sb, esel_d[:])
            ltri = pp.tile([P, P], F32, tag="ltri", name="ltri")
            nc.sync.dma_start(ltri, ltri_d[:])

            # shared-down weights (needed mid-kernel)
            sd_sb = pp.tile([P, ST, H], BF16, tag="sd", name="sd_sb")
            nc.sync.dma_start(sd_sb, sdb_d[:])

            # expert weight slabs (streamed; consumed by the g/u loop below)
            eg_tiles, eu_tiles = [], []
            for ib in range(4):
                eg_sl = wp.tile([P, HO, 512], BF16, tag="eg", name="eg_sl")
                nc.sync.dma_start(eg_sl, egb_d[:, :, ib * 512:(ib + 1) * 512])
                eu_sl = wp.tile([P, HO, 512], BF16, tag="eu", name="eu_sl")
                nc.sync.dma_start(eu_sl, eub_d[:, :, ib * 512:(ib + 1) * 512])
                eg_tiles.append(eg_sl)
                eu_tiles.append(eu_sl)
            ed_tiles = []
            for hb in range(4):
                ed_sl = wp.tile([P, IT, 256], BF16, tag="ed", name="ed_sl")
                nc.sync.dma_start(ed_sl, edb_d[:, :, hb * 256:(hb + 1) * 256])
                ed_tiles.append(ed_sl)

            # ---- helpers ----
            gsT = pp.tile([P, ST, T], BF16, tag="gsT", name="gsT")
            L_sb = pp.tile([P, TT, E], F32, tag="L", name="L_sb")

            ps_q_ctx = tc.tile_pool(name="ps_q", bufs=4, space="PSUM")
            ps_q = ps_q_ctx.__enter__()

            def shared_gu(si, q):
                qsl = slice(q * QF, (q + 1) * QF)
                psg = ps_q.tile([P, QF], F32, tag="ps_q", name="psg_s")
                for ko in range(HO):
                    nc.tensor.matmul(psg, sg_sb[:, ko, si * P:(si + 1) * P],
                                     xtb[:, ko, qsl],
                                     start=(ko == 0), stop=(ko == HO - 1))
                psu = ps_q.tile([P, QF], F32, tag="ps_q", name="psu_s")
                for ko in range(HO):
                    nc.tensor.matmul(psu, su_sb[:, ko, si * P:(si + 1) * P],
                                     xtb[:, ko, qsl],
                                     start=(ko == 0), stop=(ko == HO - 1))
                sil = op.tile([P, QF], BF16, tag="sil", name="sil_s", bufs=2)
                nc.scalar.activation(sil, psg, AF.Silu)
                nc.vector.tensor_tensor(gsT[:, si, qsl], sil, psu, ALU.mult)

            def router(tt):
                psL = ps_s.tile([P, E], F32, tag="sm", name="psL")
                tsl = slice(tt * P, (tt + 1) * P)
                k = 0
                for (xs, ws) in ((xtb, rwb), (xtb, rwr), (xtr, rwb)):
                    for ko in range(HO):
                        nc.tensor.matmul(psL, xs[:, ko, tsl], ws[:, ko, :],
                                         start=(k == 0), stop=(k == 3 * HO - 1))
                        k += 1
                nc.vector.tensor_copy(L_sb[:, tt, :], psL)

            # ---- phase 1: shared quarters 0-1 (slab 0), router h0 ----
            shared_gu(0, 0)
            shared_gu(1, 0)
            for tt in range(4):
                router(tt)
            shared_gu(0, 1)
            shared_gu(1, 1)
            for tt in range(4, 8):
                router(tt)
            shared_gu(2, 0)
            shared_gu(3, 0)

            # ---- top-1 combine: mask m_sb and weight combw, both [t_p, tt] --
            maxc = pp.tile([P, TT], F32, tag="maxc", name="maxc")
            nc.vector.reduce_max(maxc, L_sb, axis=AX.X)
            w_sb = pp.tile([P, TT], F32, tag="wsb", name="w_sb")
            nc.scalar.activation(w_sb, maxc, AF.Sigmoid)
            eq = pp.tile([P, TT, E], F32, tag="eq", name="eq")
            nc.vector.tensor_tensor(eq, L_sb,
                                    maxc[:, :, None].to_broadcast([P, TT, E]),
                                    ALU.is_equal)
            nc.vector.tensor_tensor(eq, eq,
                                    esel_sb[:, None, :].to_broadcast([P, TT, E]),
                                    ALU.mult)
            m_sb = pp.tile([P, TT], F32, tag="m", name="m_sb")
            nc.vector.reduce_sum(m_sb, eq, axis=AX.X)
            combw = pp.tile([P, TT], F32, tag="combw", name="combw")
            nc.vector.tensor_tensor(combw, m_sb, w_sb, ALU.mult)

            # ---- capacity slots: slot[t] = #selected tokens before t ----
            ps_cs = ps_s.tile([P, TT], F32, tag="sm", name="ps_cs")
            nc.tensor.matmul(ps_cs, ltri, m_sb, start=True, stop=True)
            ps_sm2 = ps_s.tile([TT, 1], F32, tag="sm", name="ps_sm2")
            nc.tensor.matmul(ps_sm2, m_sb, onescol, start=True, stop=True)
            sumsT = pp.tile([TT, 1], F32, tag="sumsT", name="sumsT")
            nc.vector.tensor_copy(sumsT, ps_sm2)
            LS = pp.tile([TT, TT], F32, tag="LS", name="LS")
            nc.vector.tensor_tensor(LS, ltri[:TT, :TT],
                                    sumsT.to_broadcast([TT, TT]), ALU.mult)
            slot = pp.tile([P, TT], F32, tag="slot", name="slot")
            nc.vector.tensor_copy(slot, ps_cs)
            ps_off = ps_s.tile([P, TT], F32, tag="sm", name="ps_off")
            nc.tensor.matmul(ps_off, allones8, LS, start=True, stop=True)

            shared_gu(2, 1)

            nc.vector.tensor_tensor(slot, slot, ps_off, ALU.add)
            slotm = pp.tile([P, TT], F32, tag="slotm", name="slotm")
            nc.vector.tensor_tensor(slotm, slot, m_sb, ALU.mult)
            inv = pp.tile([P, TT], F32, tag="inv", name="inv")
            nc.vector.tensor_scalar(inv, m_sb, -BIG, BIG, ALU.mult, ALU.add)
            nc.vector.tensor_tensor(slotm, slotm, inv, ALU.add)

            # ---- wrapped gather/scatter index list (int16) ----
            # idx[j] lives at [j%16, j//16]; factor (slot==j) as
            # (slot%16 == j%16) x (slot//16 == j//16), so idx = lhs16^T @ rhsI.
            div16 = pp.tile([P, TT], F32, tag="div16", name="div16")
            tmp16 = pp.tile([P, TT, 16], F32, tag="tmp16", name="tmp16")
            nc.vector.tensor_tensor(tmp16,
                                    slotm[:, :, None].to_broadcast([P, TT, 16]),
                                    iotag[:, None, :].to_broadcast([P, TT, 16]),
                                    ALU.is_ge)
            nc.vector.reduce_sum(div16, tmp16, axis=AX.X)
            mod16 = pp.tile([P, TT], F32, tag="mod16", name="mod16")
            nc.vector.tensor_scalar(mod16, div16, -16.0, 0.0, ALU.mult, ALU.add)
            nc.vector.tensor_tensor(mod16, slotm, mod16, ALU.add)
            lhs16 = pp.tile([P, TT, P], F32, tag="lhs16", name="lhs16")
            nc.vector.tensor_tensor(lhs16,
                                    mod16[:, :, None].to_broadcast([P, TT, P]),
                                    iotam[:, None, :].to_broadcast([P, TT, P]),
                                    ALU.is_equal)
            rhsI = pp.tile([P, TT, 16], F32, tag="rhsI", name="rhsI")
            nc.vector.tensor_tensor(rhsI,
                                    div16[:, :, None].to_broadcast([P, TT, 16]),
                                    iotac[:, None, :16].to_broadcast([P, TT, 16]),
                                    ALU.is_equal)
            nc.vector.tensor_tensor(rhsI, rhsI,
                                    iotat[:, :, None].to_broadcast([P, TT, 16]),
                                    ALU.mult)

            shared_gu(3, 1)

            # idx list is read per 16-partition stripe by each of the 8 Q7
            # cores; lhs16 replicates the slot%16 match across all 128
            # partitions so the matmul output is already stripe-replicated.
            ps_idx = ps_s.tile([P, 16], F32, tag="sm", name="ps_idx")
            for tt in range(TT):
                nc.tensor.matmul(ps_idx, lhs16[:, tt, :], rhsI[:, tt, :],
                                 start=(tt == 0), stop=(tt == TT - 1))
            idx16 = pp.tile([P, CPAD // 16], I16, tag="idx16", name="idx16")
            nc.vector.tensor_copy(idx16, ps_idx)
            # scatter index list: entry 16+j targets row tok[j]+1 (the SWDGE
            # scatter drops the first entry and row 0 is the trash row).
            # slot+16 keeps the same mod16, so lhs16 is reused; div16+1
            # shifts the column comparison by one.
            rhsIp = pp.tile([P, TT, 16], F32, tag="rhsIp", name="rhsIp")
            nc.vector.tensor_tensor(rhsIp,
                                    div16[:, :, None].to_broadcast([P, TT, 16]),
                                    iotasm1[:, None, :].to_broadcast([P, TT, 16]),
                                    ALU.is_equal)
            nc.vector.tensor_tensor(rhsIp, rhsIp,
                                    iotat1[:, :, None].to_broadcast([P, TT, 16]),
                                    ALU.mult)
            ps_idx2 = ps_s.tile([P, 16], F32, tag="sm", name="ps_idx2")
            for tt in range(TT):
                nc.tensor.matmul(ps_idx2, lhs16[:, tt, :], rhsIp[:, tt, :],
                                 start=(tt == 0), stop=(tt == TT - 1))
            idx16s = pp.tile([P, CPAD // 16], I16, tag="idx16s",
                             name="idx16s")
            nc.vector.tensor_copy(idx16s, ps_idx2)

            # ---- per-slot combine weight row cwB[p, j] = combw[tok[j]] ----
            perm = pp.tile([P, TT, CPAD], BF16, tag="perm", name="perm")
            nc.vector.tensor_tensor(
                perm, slotm[:, :, None].to_broadcast([P, TT, CPAD]),
                iotac[:, None, :].to_broadcast([P, TT, CPAD]), ALU.is_equal)
            nc.vector.tensor_tensor(
                perm, perm, combw[:, :, None].to_broadcast([P, TT, CPAD]),
                ALU.mult)

            shared_gu(0, 2)

            ps_cw = ps_s.tile([P, CPAD], F32, tag="sm", name="ps_cw")
            for tt in range(TT):
                nc.tensor.matmul(ps_cw, onesb, perm[:, tt, :],
                                 start=(tt == 0), stop=(tt == TT - 1))
            cwB = pp.tile([P, CPAD], BF16, tag="cwB", name="cwB")
            nc.vector.tensor_copy(cwB, ps_cw)

            # ---- token gather (DMA, transposed): xeT[p, ho, j] ----
            xeT = pp.tile([P, HO, CPAD], BF16, tag="xeT", name="xeT")
            nc.gpsimd.dma_gather(xeT[:], xnat_d[:], idx16[:], CPAD, CPAD, H,
                                 transpose=True)
            xeTs = pp.tile([P, HO, CPAD], BF16, tag="xeTs", name="xeTs")
            for ko in range(HO):
                nc.vector.tensor_tensor(xeTs[:, ko, :], xeT[:, ko, :], cwB,
                                        ALU.mult)
            if DBG:
                nc.sync.dma_start(dbg_xe_d[:], xeT)
                nc.sync.dma_start(dbg_cw_d[:], cwB)
                nc.sync.dma_start(dbg_idx_d[:], idx16)

            # remaining shared quarters
            shared_gu(1, 2)
            shared_gu(2, 2)
            shared_gu(3, 2)
            shared_gu(0, 3)
            shared_gu(1, 3)
            shared_gu(2, 3)
            shared_gu(3, 3)
            ps_q_ctx.__exit__(None, None, None)
            ps_g_ctx = tc.tile_pool(name="ps_gu", bufs=4, space="PSUM")
            ps_g = ps_g_ctx.__enter__()

            # ---- shared down (tokens on partitions) + out partial ----
            def shared_down(tt):
                o_t = op.tile([P, H], BF16, tag="ot", name="o_t")
                for hh in range(NH):
                    psD = ps_b.tile([P, NF], F32, tag="ps_b", name="psD")
                    for sk in range(ST):
                        nc.tensor.matmul(psD,
                                         gsT[:, sk, tt * P:(tt + 1) * P],
                                         sd_sb[:, sk, hh * NF:(hh + 1) * NF],
                                         start=(sk == 0), stop=(sk == ST - 1))
                    if hh == 0:
                        nc.scalar.activation(o_t[:, hh * NF:(hh + 1) * NF],
                                             psD, AF.Copy)
                    else:
                        nc.vector.tensor_copy(o_t[:, hh * NF:(hh + 1) * NF],
                                              psD)
                nc.sync.dma_start(outsh_d[tt * P:(tt + 1) * P, :], o_t)

            for tt in range(6):
                shared_down(tt)

            # ---- expert gate/up at capacity C -> gTe[i_p, it, j] ----
            gTe = pp.tile([P, IT, C], BF16, tag="gTe", name="gTe")
            for ib in range(4):
                for a in range(4):
                    it = ib * 4 + a
                    psg = ps_g.tile([P, C], F32, tag="ps_g", name="psg_e")
                    for ko in range(HO):
                        nc.tensor.matmul(psg,
                                         eg_tiles[ib][:, ko,
                                                      a * P:(a + 1) * P],
                                         xeTs[:, ko, 0:C],
                                         start=(ko == 0), stop=(ko == HO - 1))
                    psu = ps_g.tile([P, C], F32, tag="ps_g", name="psu_e")
                    for ko in range(HO):
                        nc.tensor.matmul(psu,
                                         eu_tiles[ib][:, ko,
                                                      a * P:(a + 1) * P],
                                         xeTs[:, ko, 0:C],
                                         start=(ko == 0), stop=(ko == HO - 1))
                    sil = op.tile([P, C], BF16, tag="sil_e", name="sil_e",
                                  bufs=2)
                    nc.scalar.activation(sil, psg, AF.Silu)
                    nc.vector.tensor_tensor(gTe[:, it, :], sil, psu, ALU.mult)

            for tt in range(6, 8):
                shared_down(tt)

            # ---- expert down [h_p, j], transpose to [j_p, h], scatter ----
            scT = pp.tile([P, CB, H], BF16, tag="scT", name="scT")
            for hb in range(4):
                for hj in range(2):
                    ho = hb * 2 + hj
                    psd = ps_g.tile([P, C], F32, tag="ps_g", name="psd")
                    for ik in range(IT):
                        nc.tensor.matmul(psd,
                                         ed_tiles[hb][:, ik,
                                                      hj * P:(hj + 1) * P],
                                         gTe[:, ik, :],
                                         start=(ik == 0), stop=(ik == IT - 1))
                    re_sb = op.tile([P, CE], BF16, tag="re", name="re_sb")
                    nc.vector.memset(re_sb[:, 0:16], 0.0)
                    nc.scalar.activation(re_sb[:, 16:16 + C], psd, AF.Copy)
                    for cb in range(CB):
                        cq = min(P, CE - cb * P)
                        ps_tr = ps_s.tile([P, P], BF16, tag="sm",
                                          name="ps_tr")
                        nc.tensor.transpose(ps_tr[0:cq, :],
                                            re_sb[:, cb * P:cb * P + cq],
                                            identb)
                        nc.scalar.activation(
                            scT[0:cq, cb, ho * P:(ho + 1) * P],
                            ps_tr[0:cq, :], AF.Copy)

            if DBG:
                nc.sync.dma_start(dbg_sc_d[:, 0:CB, :], scT)
            nc.gpsimd.dma_scatter_add(routed_d[:], scT[:], idx16s[:, 0:CI],
                                      CE, CE, H)
            ps_g_ctx.__exit__(None, None, None)

    nc.compile()
    return nc


@functools.lru_cache(maxsize=2)
def _get_nc_for(C):
    return _build_nc(C)


_LAST_NC = None


def _get_nc():
    return _LAST_NC


def _pick_capacity(x, rw):
    logits = x.astype(np.float32) @ rw.astype(np.float32).T
    loads = np.bincount(logits.argmax(1), minlength=E)
    c = int(loads.max()) + 8
    c = (c + 15) // 16 * 16
    return max(32, min(CPAD - 16, c))


def _rearr(w, nblk):
    # [(k p), cols] -> [p, k, cols]
    return np.ascontiguousarray(
        w.reshape(nblk, P, -1).transpose(1, 0, 2))


def _make_in_maps(inputs):
    f32 = lambda v: np.asarray(v, dtype=np.float32)
    x = f32(inputs["hidden_states"])
    rw = f32(inputs["router_weight"])
    sg = f32(inputs["shared_gate"])
    su = f32(inputs["shared_up"])
    sd = f32(inputs["shared_down"])
    eg = f32(inputs["expert_gate"])
    eu = f32(inputs["expert_up"])
    ed = f32(inputs["expert_down"])

    xT = np.ascontiguousarray(x.T)                    # [H, T]
    xTb = xT.astype(BF)
    xTr = (xT - xTb.astype(np.float32)).astype(BF)
    rwT = np.ascontiguousarray(rw.T)                  # [H, E]
    rwTb = rwT.astype(BF)
    rwTr = (rwT - rwTb.astype(np.float32)).astype(BF)

    iotac = np.tile(np.arange(CPAD, dtype=np.float32), (P, 1))
    iotag = np.tile(16.0 * np.arange(1, 17, dtype=np.float32), (P, 1))
    iotam = np.tile((np.arange(P) % 16).astype(np.float32), (P, 1))
    iotasm1 = np.tile(np.arange(-1, 15, dtype=np.float32), (P, 1))
    iotat = (np.arange(P, dtype=np.float32)[:, None]
             + P * np.arange(TT, dtype=np.float32)[None, :])
    ltri = np.triu(np.ones((P, P), dtype=np.float32), 1)

    common = {
        "xtb": _rearr(xTb, HO),
        "xtr": _rearr(xTr, HO),
        "xnat": np.ascontiguousarray(x.astype(BF)),
        "rwb": _rearr(rwTb, HO),
        "rwr": _rearr(rwTr, HO),
        "iotac": iotac,
        "iotag": np.ascontiguousarray(iotag),
        "iotam": np.ascontiguousarray(iotam),
        "iotasm1": np.ascontiguousarray(iotasm1),
        "iotat1": np.ascontiguousarray(iotat + 1.0),
        "iotat": np.ascontiguousarray(iotat),
        "ltri": ltri,
    }
    in_maps = []
    for c in range(NCORES):
        esel = np.zeros((P, E), dtype=np.float32)
        esel[:, c] = 1.0
        sl = slice(c * SIS, (c + 1) * SIS)
        in_maps.append({
            **common,
            "esel": esel,
            "sgb": _rearr(sg[:, sl].astype(BF), HO),
            "sub": _rearr(su[:, sl].astype(BF), HO),
            "sdb": _rearr(sd[sl, :].astype(BF), ST),
            "egb": _rearr(eg[c].astype(BF), HO),
            "eub": _rearr(eu[c].astype(BF), HO),
            "edb": _rearr(ed[c].astype(BF), IT),
        })
    return in_maps


def _run(inputs, trace=False):
    global _LAST_NC
    from concourse.bass_utils import run_bass_kernel_spmd
    C = _pick_capacity(np.asarray(inputs["hidden_states"]),
                       np.asarray(inputs["router_weight"]))
    nc = _get_nc_for(C)
    _LAST_NC = nc
    in_maps = _make_in_maps(inputs)
    res = run_bass_kernel_spmd(nc, in_maps, core_ids=list(range(NCORES)),
                               trace=trace)
    acc = np.zeros((T, H), dtype=np.float32)
    for r in res.results:
        acc += r["outsh"].astype(np.float32)
        acc += r["routed"][1:].astype(np.float32)
    return acc, res


def kernel(**inputs) -> np.ndarray:
    out, _ = _run(inputs, trace=False)
    return out
